# revision 22
# baseline (speedup 1.0000x reference)
"""Trainium2 Bass kernel for nn_JointLearner (retrieval_knn).

Math summary (exact algebraic collapse of the reference):
  - _convolve(X, pos) is linear in X:  conv_out = X @ C_pos^T, so
    _convolve(X, pos) = X @ M_pos^T + fc_b with M_pos = fc_W @ C_pos (200x200).
  - e5 = entity[:NL] @ M4^T + fc_b  =>  hypeScores = p @ e5^T
      = (p @ [M4 | fc_b]) @ [entity[:NL] | 1]^T  = Z_ext @ E_ext^T.
  - ontScores[b,j] = sum_k |s[b,k] - ontology[j,k]|  (L1 distance),
    s = ontology[type_idx] + relu(relation[rel2_idx] @ pR_W^T + pR_b).

Sharding: ontology rows (NO) and entity rows (NL) are split 8 ways; every
core computes the full batch path (tiny) plus its shard of both score
matrices. Outputs are concatenated along the candidate axis on the host.

On-device layout: everything is transposed so the contraction/candidate
dims sit on SBUF partitions. The L1 reduction over OD runs on the tensor
engine: per batch row b a fused DVE/ACT op produces |O^T - s_b| and a
ones-column matmul (one-hot lhsT slice) accumulates its column sums into
PSUM row b.
"""

import os
from contextlib import ExitStack

import numpy as np
import ml_dtypes

import concourse.bacc as bacc
import concourse.tile as tile
import concourse.mybir as mybir
from concourse.bass_utils import run_bass_kernel_spmd

F32 = mybir.dt.float32
BF16 = mybir.dt.bfloat16
BF16_NP = ml_dtypes.bfloat16

NCORES = 8
B = 256
D = 200
OD = 100
NO = 5000
NL = 50000
NOS = NO // NCORES      # 625
NLS = NL // NCORES      # 6250
KE = D + 1              # 201  (entity dim + bias row)
OUT_CH = 2
FW = 10
STRIDE = 5
WP = (D - FW) // STRIDE + 1   # 39
FC_LEN = OUT_CH * WP          # 78

_CACHE = {}


def _conv_matrix(kvec):
    """C_pos [FC_LEN, D] with conv_out[n] = C_pos @ x[n]."""
    k = np.asarray(kvec, np.float32).reshape(OUT_CH, FW)
    C = np.zeros((FC_LEN, D), np.float32)
    for o in range(OUT_CH):
        for w in range(WP):
            C[o * WP + w, STRIDE * w:STRIDE * w + FW] = k[o]
    return C


def _build_program():
    nc = bacc.Bacc(
        "TRN2",
        target_bir_lowering=False,
        debug=False,
        enable_asserts=False,
        num_devices=NCORES,
    )

    def din(name, shape, dt=F32):
        return nc.dram_tensor(name, shape, dt, kind="ExternalInput").ap()

    # per-core shards
    o_t = din("o_t", [OD, NOS + 1], BF16)      # ontology shard^T, zero-padded col
    osum2 = din("osum2", [2, NOS + 1], BF16)   # -sum_k o[j,k], bf16 hi+lo split
    negssum = din("negssum", [128, 2])         # +sum_k s[b,k], per b-block column
    e_t = din("e_t", [KE, NLS], BF16)          # [entity shard | 1]^T
    # packed replicated tensors (host-side concat; fewer, bigger DMAs)
    crit = din("crit", [D, B + OD])            # rel2^T | pR_W^T
    bigo = din("bigo", [OD, D + B + 1])        # proj_W^T | typeSub^T | pR_b
    bige = din("bige", [D, 4 * B])             # rel^T | eu^T | ed^T | eh^T
    bigm = din("bigm", [D, 4 * D + KE + 2])    # M1t|M2t|M3t|M5t|M4e|pjb|fcb
    ont_out = nc.dram_tensor("ont_out", [B, NOS], F32, kind="ExternalOutput").ap()
    hyp_out = nc.dram_tensor("hyp_out", [B, NLS], F32, kind="ExternalOutput").ap()

    with tile.TileContext(nc) as tc, ExitStack() as ctx:
        const = ctx.enter_context(tc.tile_pool(name="const", bufs=1))
        work = ctx.enter_context(tc.tile_pool(name="work", bufs=2))
        tbp = ctx.enter_context(tc.tile_pool(name="tbp", bufs=8))
        ps_sp = ctx.enter_context(tc.tile_pool(name="ps_sp", bufs=2, space="PSUM"))
        ps_ont = ctx.enter_context(tc.tile_pool(name="ps_ont", bufs=1, space="PSUM"))
        ps_hyp = ctx.enter_context(tc.tile_pool(name="ps_hyp", bufs=3, space="PSUM"))

        dma = nc.sync.dma_start
        def load(name, ap, dt=F32):
            p, f = ap.shape
            t = const.tile([p, f], dt, name=name, tag=name)
            dma(out=t, in_=ap)
            return t

        # ---- constant loads (packed; critical path first) -------------------
        crit_a = const.tile([128, B + OD], F32, tag="crit_a")
        crit_b = const.tile([D - 128, B + OD], F32, tag="crit_b")
        dma(out=crit_a, in_=crit[0:128, :])
        dma(out=crit_b, in_=crit[128:D, :])
        bigo_sb = const.tile([OD, D + B + 1], F32, tag="bigo_sb")
        dma(out=bigo_sb, in_=bigo)
        bigm_a = const.tile([128, 4 * D + KE + 2], F32, tag="bigm_a")
        bigm_b = const.tile([D - 128, 4 * D + KE + 2], F32, tag="bigm_b")
        dma(out=bigm_a, in_=bigm[0:128, :])
        dma(out=bigm_b, in_=bigm[128:D, :])
        bige_a = const.tile([128, 4 * B], F32, tag="bige_a")
        bige_b = const.tile([D - 128, 4 * B], F32, tag="bige_b")
        nc.gpsimd.dma_start(out=bige_a, in_=bige[0:128, :])
        nc.gpsimd.dma_start(out=bige_b, in_=bige[128:D, :])
        ot_sb = const.tile([OD, NOS + 1], BF16, tag="ot_sb")
        dma(out=ot_sb, in_=o_t)
        # bulk entity shard on the SWDGE queue so it never blocks HWDGE
        et_a = const.tile([128, NLS], BF16, tag="et_a")
        et_b = const.tile([KE - 128, NLS], BF16, tag="et_b")
        nc.gpsimd.dma_start(out=et_a, in_=e_t[0:128, :])
        nc.gpsimd.dma_start(out=et_b, in_=e_t[128:KE, :])

        rel2t_a, prwt_a = crit_a[:, 0:B], crit_a[:, B:B + OD]
        rel2t_b, prwt_b = crit_b[:, 0:B], crit_b[:, B:B + OD]
        pjwt_sb = bigo_sb[:, 0:D]
        tst = bigo_sb[:, D:D + B]
        prb_sb = bigo_sb[:, D + B:D + B + 1]
        relt_a, eut_a = bige_a[:, 0:B], bige_a[:, B:2 * B]
        edt_a, eht_a = bige_a[:, 2 * B:3 * B], bige_a[:, 3 * B:4 * B]
        relt_b, eut_b = bige_b[:, 0:B], bige_b[:, B:2 * B]
        edt_b, eht_b = bige_b[:, 2 * B:3 * B], bige_b[:, 3 * B:4 * B]
        m1t_a, m2t_a = bigm_a[:, 0:D], bigm_a[:, D:2 * D]
        m3t_a, m5t_a = bigm_a[:, 2 * D:3 * D], bigm_a[:, 3 * D:4 * D]
        m4e_a = bigm_a[:, 4 * D:4 * D + KE]
        pjb_a = bigm_a[:, 4 * D + KE:4 * D + KE + 1]
        fcb_a = bigm_a[:, 4 * D + KE + 1:4 * D + KE + 2]
        m1t_b, m2t_b = bigm_b[:, 0:D], bigm_b[:, D:2 * D]
        m3t_b, m5t_b = bigm_b[:, 2 * D:3 * D], bigm_b[:, 3 * D:4 * D]
        m4e_b = bigm_b[:, 4 * D:4 * D + KE]
        pjb_b = bigm_b[:, 4 * D + KE:4 * D + KE + 1]
        fcb_b = bigm_b[:, 4 * D + KE + 1:4 * D + KE + 2]

        # one-hot matrix for the L1 row-reduce: column 128 is all-2s
        # (|d| = 2*relu(d) - d; the -sum(d) part is Osum_j - Ssum_b)
        oh = const.tile([OD, 2 * 128], BF16, tag="oh")
        nc.gpsimd.memset(oh, 0.0)
        nc.gpsimd.memset(oh[:, 128:129], 2.0)
        osum_sb = const.tile([2, NOS + 1], BF16, tag="osum_sb")
        dma(out=osum_sb, in_=osum2)
        nss_sb = load("nss_sb", negssum)
        ones2 = const.tile([2, 128], BF16, tag="ones2")
        nc.gpsimd.memset(ones2, 1.0)

        mm = nc.tensor.matmul
        AF = mybir.ActivationFunctionType
        use_f32r = os.environ.get("KERNEL_F32R", "0") == "1"

        def mmr(out, l, r, **kw):
            if use_f32r:
                l = l.bitcast(mybir.dt.float32r)
                r = r.bitcast(mybir.dt.float32r)
            mm(out, l, r, **kw)

        # ---- s^T = typeSub^T + relu(pR_W @ rel2^T + pR_b) -------------------
        ps_s = ps_sp.tile([OD, B], F32, tag="sp")
        mmr(ps_s, prwt_a, rel2t_a, start=True, stop=False)
        mmr(ps_s, prwt_b, rel2t_b, start=False, stop=True)
        sT = const.tile([OD, B], F32, tag="sT")
        nc.scalar.activation(sT, ps_s, AF.Relu, bias=prb_sb[:, 0:1], scale=1.0)
        nc.vector.tensor_add(sT, sT, tst)

        # ---- batch path: conv branches, p, Z_ext ----------------------------
        def affine(tagbase, mt_a, mt_b, rhs_a, rhs_b, bias_a, bias_b):
            outs = []
            for ci, (sl, psz, bias) in enumerate(((slice(0, 128), 128, bias_a),
                                                  (slice(128, D), D - 128, bias_b))):
                ps = ps_sp.tile([psz, B], F32, name=f"ps_{tagbase}{ci}", tag="sp")
                mmr(ps, mt_a[:, sl], rhs_a, start=True, stop=False)
                mmr(ps, mt_b[:, sl], rhs_b, start=False, stop=True)
                o = work.tile([psz, B], F32, name=f"{tagbase}{ci}", tag=f"{tagbase}{ci}")
                nc.scalar.activation(o, ps, AF.Identity, bias=bias[:, 0:1], scale=1.0)
                outs.append(o)
            return outs

        # ttpre^T = relu(proj_W @ typeSub^T + proj_b)   (single-K matmuls)
        tt_pre = []
        for ci, (sl, psz, bias) in enumerate(((slice(0, 128), 128, pjb_a),
                                              (slice(128, D), D - 128, pjb_b))):
            ps = ps_sp.tile([psz, B], F32, name=f"ps_ttp{ci}", tag="sp")
            mmr(ps, pjwt_sb[:, sl], tst, start=True, stop=True)
            o = work.tile([psz, B], F32, name=f"ttp{ci}", tag=f"ttp{ci}")
            nc.scalar.activation(o, ps, AF.Relu, bias=bias[:, 0:1], scale=1.0)
            tt_pre.append(o)

        u_ab = affine("cu", m1t_a, m1t_b, eut_a, eut_b, fcb_a, fcb_b)
        d_ab = affine("cd", m2t_a, m2t_b, edt_a, edt_b, fcb_a, fcb_b)
        h_ab = affine("ch", m3t_a, m3t_b, eht_a, eht_b, fcb_a, fcb_b)
        t_ab = affine("ct", m5t_a, m5t_b, tt_pre[0], tt_pre[1], fcb_a, fcb_b)

        p_ab = []
        for c, rel_c in ((0, relt_a), (1, relt_b)):
            psz = 128 if c == 0 else D - 128
            pt = work.tile([psz, B], F32, name=f"pt{c}", tag=f"pt{c}")
            nc.vector.tensor_mul(pt, rel_c, u_ab[c])
            nc.vector.tensor_mul(pt, pt, d_ab[c])
            nc.vector.tensor_mul(pt, pt, h_ab[c])
            nc.vector.tensor_mul(pt, pt, t_ab[c])
            p_ab.append(pt)

        zT = []
        for ci, (sl, psz) in enumerate(((slice(0, 128), 128), (slice(128, KE), KE - 128))):
            ps = ps_sp.tile([psz, B], F32, name=f"ps_z{ci}", tag="sp")
            mmr(ps, m4e_a[:, sl], p_ab[0], start=True, stop=False)
            mmr(ps, m4e_b[:, sl], p_ab[1], start=False, stop=True)
            z = const.tile([psz, B], BF16, name=f"zT{ci}", tag=f"zT{ci}")
            nc.vector.tensor_copy(z, ps)
            zT.append(z)

        # ---- hypeScores = Z_ext @ E_ext^T (1MB staged stores) -------------
        ntiles = [(i * 512, min(512, NLS - i * 512)) for i in range((NLS + 511) // 512)]
        for bc in range(2):
            zza = zT[0][:, bc * 128:(bc + 1) * 128]
            zzb = zT[1][:, bc * 128:(bc + 1) * 128]
            hs = None
            for it, (n0, nsz) in enumerate(ntiles):
                if hs is None:
                    g0 = n0
                    hs = work.tile([128, 4 * 512], F32, name="hs", tag="hs")
                ph = ps_hyp.tile([128, 512], F32, name="ph", tag="h")
                mm(ph[:, 0:nsz], zza, et_a[:, n0:n0 + nsz], start=True, stop=False)
                mm(ph[:, 0:nsz], zzb, et_b[:, n0:n0 + nsz], start=False, stop=True)
                off = n0 - g0
                if it % 2 == 0:
                    nc.vector.tensor_copy(hs[:, off:off + nsz], ph[:, 0:nsz])
                else:
                    nc.scalar.copy(hs[:, off:off + nsz], ph[:, 0:nsz])
                if it % 4 == 3 or it == len(ntiles) - 1:
                    w = n0 + nsz - g0
                    dma(out=hyp_out[bc * 128:(bc + 1) * 128, g0:g0 + w],
                        in_=hs[:, 0:w])
                    hs = None

        # ---- ontScores: 256 abs-diff tiles + one-hot ones-matmul reduce -----
        for blk in range(2):
            po1 = ps_ont.tile([128, 512], F32, tag="o1")
            po2 = ps_ont.tile([128, NOS - 512], F32, tag="o2")
            # seed every PSUM row with Osum_j (ones-matmul, bf16 hi+lo rows)
            mm(po1, ones2, osum_sb[:, 0:512], start=True, stop=False)
            mm(po2, ones2, osum_sb[:, 512:NOS], start=True, stop=False)
            for lb in range(128):
                b = blk * 128 + lb
                tb = tbp.tile([OD, NOS + 1], BF16, tag="tb")
                # relu(o - s) on DVE: (o - s) then max(., 0), one fused op
                nc.vector.tensor_scalar(
                    out=tb, in0=ot_sb,
                    scalar1=sT[:, b:b + 1], scalar2=0.0,
                    op0=mybir.AluOpType.subtract, op1=mybir.AluOpType.max,
                )
                lw = oh[:, 128 - lb:256 - lb]
                mm(po1, lw, tb[:, 0:512], start=False, stop=(lb == 127))
                mm(po2, lw, tb[:, 512:NOS], start=False, stop=(lb == 127))
            onts = work.tile([128, NOS], F32, tag="onts")
            nc.scalar.activation(onts[:, 0:512], po1, AF.Identity,
                                 bias=nss_sb[:, blk:blk + 1], scale=1.0)
            nc.scalar.activation(onts[:, 512:NOS], po2, AF.Identity,
                                 bias=nss_sb[:, blk:blk + 1], scale=1.0)
            dma(out=ont_out[blk * 128:(blk + 1) * 128, :], in_=onts)

    nc.compile()
    return nc


def _prep_inputs(inputs):
    """Host-side gather/transpose/precompute. Returns per-core in_maps."""
    f32 = lambda x: np.ascontiguousarray(np.asarray(x, np.float32))
    entity = f32(inputs["entity_emb"])
    ontology = f32(inputs["ontology_emb"])
    relation = f32(inputs["relation_emb"])
    fc_W = f32(inputs["fc_W"])          # [D, FC_LEN]
    fc_b = f32(inputs["fc_b"])          # [D]
    fc2_W = f32(inputs["fc2_W"])        # [FC1_LEN, MAX_ARITY+1]
    fc2_b = f32(inputs["fc2_b"])        # [FC1_LEN]

    M = {}
    for pos in (1, 2, 3, 4, 5):
        M[pos] = fc_W @ _conv_matrix(fc2_W[:, pos] + fc2_b)   # [D, D]

    idx = lambda k: np.asarray(inputs[k]).astype(np.int64)
    rel_gT = f32(relation[idx("rel_idx")].T)
    rel2_gT = f32(relation[idx("rel2_idx")].T)
    eu_T = f32(entity[idx("user_idx")].T)
    ed_T = f32(entity[idx("day_idx")].T)
    eh_T = f32(entity[idx("hour_idx")].T)
    ts_T = f32(ontology[idx("type_idx")].T)

    # +Ssum[b] correction, laid out [128, 2] (partition = b within block)
    s_host = ts_T.T + np.maximum(rel2_gT.T @ f32(inputs["pR_W"]).T
                                 + f32(inputs["pR_b"]), 0.0)      # [B, OD]
    scorr = np.ascontiguousarray(s_host.sum(1).reshape(2, 128).T.astype(np.float32))

    common = {
        "negssum": scorr,
        "crit": np.ascontiguousarray(np.concatenate(
            [rel2_gT, f32(np.asarray(inputs["pR_W"]).T)], axis=1)),
        "bigo": np.ascontiguousarray(np.concatenate(
            [f32(np.asarray(inputs["proj_W"]).T), ts_T,
             f32(inputs["pR_b"]).reshape(OD, 1)], axis=1)),
        "bige": np.ascontiguousarray(np.concatenate(
            [rel_gT, eu_T, ed_T, eh_T], axis=1)),
        "bigm": np.ascontiguousarray(np.concatenate(
            [f32(M[1].T), f32(M[2].T), f32(M[3].T), f32(M[5].T),
             f32(np.concatenate([M[4], fc_b.reshape(D, 1)], axis=1)),
             f32(inputs["proj_b"]).reshape(D, 1), fc_b.reshape(D, 1)], axis=1)),
    }

    in_maps = []
    for c in range(NCORES):
        o_sh = ontology[c * NOS:(c + 1) * NOS]          # [NOS, OD]
        o_t = np.zeros((OD, NOS + 1), BF16_NP)
        o_t[:, :NOS] = o_sh.T.astype(BF16_NP)
        # -Osum_j seed, split into bf16 hi + residual lo rows
        oseed = np.zeros((2, NOS + 1), np.float32)
        oseed[0, :NOS] = -o_sh.astype(np.float32).sum(1)
        hi = oseed[0].astype(BF16_NP)
        lo = (oseed[0] - hi.astype(np.float32)).astype(BF16_NP)
        osum2 = np.stack([hi, lo]).astype(BF16_NP)
        e_sh = entity[c * NLS:(c + 1) * NLS]            # [NLS, D]
        e_t = np.empty((KE, NLS), BF16_NP)
        e_t[:D] = e_sh.T.astype(BF16_NP)
        e_t[D] = np.ones((NLS,), BF16_NP)
        m = dict(common)
        m["o_t"] = o_t
        m["osum2"] = osum2
        m["e_t"] = np.ascontiguousarray(e_t)
        in_maps.append(m)
    return in_maps


def kernel(**inputs):
    if "nc" not in _CACHE:
        _CACHE["nc"] = _build_program()
    nc = _CACHE["nc"]

    in_maps = _prep_inputs(inputs)
    trace = os.environ.get("KERNEL_TRACE", "0") == "1"
    res = run_bass_kernel_spmd(
        nc, in_maps, core_ids=list(range(NCORES)), trace=trace,
    )
    _CACHE["last_results"] = res

    ont = np.empty((B, NO), np.float32)
    hyp = np.empty((B, NL), np.float32)
    for c in range(NCORES):
        ont[:, c * NOS:(c + 1) * NOS] = res.results[c]["ont_out"]
        hyp[:, c * NLS:(c + 1) * NLS] = res.results[c]["hyp_out"]
    return ont, hyp
